# revision 13
# baseline (speedup 1.0000x reference)
"""BiFormer sparse attention on 8 Trainium2 NeuronCores.

Problem (hardcoded): B=4, N=2048, C=768, H=12, hd=64, keep=N/2=1024.
    qkv = x @ w_qkv -> q,k,v per (B,H)
    top-1024 tokens per (B,H) by ||q|| -> gather k,v
    out = softmax(clip(q @ k_sel^T * hd^-0.5, +-50)) @ v_sel
    y = clip(out @ w_proj + b_proj, +-10)

Sharding: 8 cores = 4 batches x 2 head-groups (6 heads each). Weights are
column/row-split per head-group; the two cores of a batch produce partial
projection outputs that the host sums (+bias, clip).

Device algorithm (per core), matmuls bf16 unless noted (fp32 PSUM):
  1. qT [384,2048] via Dekker 3-term bf16 split (selection needs fp32-grade
     scores; xh/xl ship pre-split from the host). kT/vT [384,2048] single
     bf16 pass, kept in fp32 SBUF for the gather. Squared q PSUM ->
     scn [128,6,16] token-major scores + score16 [16,6,128] (exact fp32
     PE transpose) for the compaction.
  2. Per-head top-1024 threshold: 6-level 16-ary bisection on the vector
     engine over scn, overlapped with the k/v matmuls on PE.
  3. Compaction instead of masking: payload = token_idx where score>=thr
     else -1 -> gpsimd sparse_gather -> 1024 indices per head ->
     PE-broadcast to all partitions -> gpsimd ap_gather pulls the selected
     columns of kT32/vT32. v_sel is PE-transposed back to key-major with a
     ones column appended (softmax denominator).
  4. Attention over the 1024 gathered keys only (half the S/exp/PV work of
     mask-based attention): S^T = k_sel(block)^T @ q^T, two heads per
     2-bank PSUM strip, one Exp per strip; out^T accumulates
     v_sel^T @ P over 8 key blocks; row 64 = denom.
  5. Normalize by reciprocal(denom), project with row-split w_proj
     interleaved into the attention stream; y ships bf16 (host sums the
     two partials in fp32, adds bias, clips).
"""
import os
import sys

sys.path.insert(0, "/opt/trn_rl_repo")

import numpy as np

import concourse.bass as bass
import concourse.mybir as mybir
from concourse import bacc
from concourse.tile import TileContext
from concourse.bass_utils import run_bass_kernel_spmd

B, N, C, H, HD = 4, 2048, 768, 12, 64
HPC = 6                  # heads per core
KEEP = N // 2            # 1024
KB = KEEP // 128         # 8 selected-key blocks
QC = N // 512            # 4 query chunks
CB = C // 128            # 6 contraction blocks
SCALE = HD ** -0.5       # 0.125
BISECT_HI = 512.0        # scores are chi2(64)-like, max ~150 << 512
BISECT_ITERS = 24        # kept for test.py compat
F32 = mybir.dt.float32
BF16 = mybir.dt.bfloat16
I16 = mybir.dt.int16
I32 = mybir.dt.int32
U32 = mybir.dt.uint32

_CACHE = {}
TRACE = False       # set True (e.g. from test.py) to capture an NTFF profile
LAST = {}           # exec_time_ns / profile info from the most recent run


def _build():
    nc = bacc.Bacc(None, target_bir_lowering=False)
    xh_d = nc.declare_dram_parameter("xh", [C, N], BF16, isOutput=False)
    xl_d = nc.declare_dram_parameter("xl", [C, N], BF16, isOutput=False)
    wqh_d = nc.declare_dram_parameter("wqh", [C, HPC * HD], BF16, isOutput=False)
    wql_d = nc.declare_dram_parameter("wql", [C, HPC * HD], BF16, isOutput=False)
    wk_d = nc.declare_dram_parameter("wk", [C, HPC * HD], BF16, isOutput=False)
    wv_d = nc.declare_dram_parameter("wv", [C, HPC * HD], BF16, isOutput=False)
    wp_d = nc.declare_dram_parameter("wp", [HPC * HD, C], BF16, isOutput=False)
    sel_d = nc.declare_dram_parameter("selmask", [HPC * HD, HPC], F32, isOutput=False)
    # consts [128, 256] f32 = I128 | J16 (J16[p, f] = 1 if f % 16 == p,
    # rows 16.. zero); identb = I64 stacked twice (bf16, for PE transposes)
    con_d = nc.declare_dram_parameter("consts", [128, 256], F32, isOutput=False)
    idb_d = nc.declare_dram_parameter("identb", [128, HD], BF16, isOutput=False)
    y_d = nc.declare_dram_parameter("y", [N, C], BF16, isOutput=True)
    thr_d = nc.declare_dram_parameter("dbg_thr", [1, HPC], F32, isOutput=True)
    sc_d = nc.declare_dram_parameter("dbg_scores", [128, HPC * 16], F32, isOutput=True)

    with TileContext(nc) as tc:
        with (
            tc.tile_pool(name="wts", bufs=1) as wts,
            tc.tile_pool(name="xc", bufs=1) as xcp,
            tc.tile_pool(name="xl", bufs=2) as xlp,
            tc.tile_pool(name="qk", bufs=1) as qkp,
            tc.tile_pool(name="kv32", bufs=1) as kvp,
            tc.tile_pool(name="sq", bufs=1) as sqp,
            tc.tile_pool(name="sel", bufs=1) as selp,
            tc.tile_pool(name="g32", bufs=1) as g32p,
            tc.tile_pool(name="sc", bufs=1) as scp,
            tc.tile_pool(name="small", bufs=1) as sml,
            tc.tile_pool(name="bis", bufs=1) as bis,
            tc.tile_pool(name="pt", bufs=6) as ptp,
            tc.tile_pool(name="outt", bufs=1) as otp,
            tc.tile_pool(name="y", bufs=2) as yp,
            tc.tile_pool(name="strip", bufs=2, space="PSUM") as pstrip,
            tc.tile_pool(name="po", bufs=4, space="PSUM") as ppo,
        ):
            # ---- batched loads; everything ships pre-cast bf16 (halves HBM
            # vs f32). DMA issue costs ~650ns each, so q-critical-path first.
            wqht = wts.tile([128, CB, HPC * HD], BF16, tag="wqh", name="wqht")
            nc.gpsimd.dma_start(out=wqht, in_=wqh_d.rearrange("(k p) m -> p k m", p=128))
            wqlt = wts.tile([128, CB, HPC * HD], BF16, tag="wql", name="wqlt")
            nc.gpsimd.dma_start(out=wqlt, in_=wql_d.rearrange("(k p) m -> p k m", p=128))
            xct, xlt = [], []
            for nb2 in range(QC):
                th = xcp.tile([128, CB, 512], BF16, tag=f"xc{nb2}", name=f"xc{nb2}")
                nc.gpsimd.dma_start(
                    out=th,
                    in_=xh_d[:, nb2 * 512:(nb2 + 1) * 512].rearrange("(k p) n -> p k n", p=128))
                xct.append(th)
                # xl is only read by the q Dekker terms of its own chunk:
                # 2 rotating bufs. Chunks 2/3 are DMA'd after the weights so
                # their buffer-release waits don't block the weight loads on
                # the gpsimd queue.
                xlt.append(xlp.tile([128, CB, 512], BF16, tag="xl", name=f"xl{nb2}"))
                if nb2 < 2:
                    nc.gpsimd.dma_start(
                        out=xlt[nb2],
                        in_=xl_d[:, nb2 * 512:(nb2 + 1) * 512].rearrange("(k p) n -> p k n", p=128))
            selmt = sml.tile([128, 3, HPC], F32, tag="selm", name="selmt")
            nc.gpsimd.dma_start(out=selmt, in_=sel_d.rearrange("(k p) m -> p k m", p=128))
            cont = wts.tile([128, 256], F32, tag="consts", name="cont")
            nc.gpsimd.dma_start(out=cont, in_=con_d[:, :])
            identb = wts.tile([128, HD], BF16, tag="identb", name="identb")
            nc.gpsimd.dma_start(out=identb, in_=idb_d[:, :])
            wkt = wts.tile([128, CB, HPC * HD], BF16, tag="wk", name="wkt")
            nc.gpsimd.dma_start(out=wkt, in_=wk_d.rearrange("(k p) m -> p k m", p=128))
            wvt = wts.tile([128, CB, HPC * HD], BF16, tag="wv", name="wvt")
            nc.gpsimd.dma_start(out=wvt, in_=wv_d.rearrange("(k p) m -> p k m", p=128))
            wpt = wts.tile([128, 3, C], BF16, tag="wp", name="wpt")
            nc.gpsimd.dma_start(out=wpt, in_=wp_d.rearrange("(k p) m -> p k m", p=128))
            for nb2 in range(2, QC):
                nc.gpsimd.dma_start(
                    out=xlt[nb2],
                    in_=xl_d[:, nb2 * 512:(nb2 + 1) * 512].rearrange("(k p) n -> p k n", p=128))

            i128 = cont[:, 0:128]
            j16 = cont[0:16, 128:256]
            selm = [selmt[:, i, :] for i in range(3)]

            ones_row = sml.tile([1, 128], F32, tag="ones_row")
            nc.vector.memset(ones_row, 1.0)
            ones_sb = sml.tile([128, 1], F32, tag="ones_sb")
            nc.vector.memset(ones_sb, 1.0)
            iotai = sml.tile([1, 16], I32, tag="iotai")
            nc.gpsimd.iota(iotai, pattern=[[1, 16]], channel_multiplier=0)
            iotaf = sml.tile([1, 16], F32, tag="iotaf")
            nc.vector.tensor_copy(iotaf, iotai)
            # valp1[p, f] = p*128 + f + 1  (token index + 1, wrap-16 layout
            # matches the score16 transpose: token = p*128 + f)
            valp1i = sml.tile([16, 128], I32, tag="valp1i")
            nc.gpsimd.iota(valp1i, pattern=[[1, 128]], base=1, channel_multiplier=128)
            valp1 = sml.tile([16, 128], F32, tag="valp1")
            nc.vector.tensor_copy(valp1, valp1i)

            qkT = [qkp.tile([128, N], BF16, tag=f"qT{mb}", name=f"qT{mb}")
                   for mb in range(3)]
            kT32 = [kvp.tile([128, N], F32, tag=f"kT32_{hp}", name=f"kT32_{hp}")
                    for hp in range(3)]
            vT32 = [kvp.tile([128, N], F32, tag=f"vT32_{hp}", name=f"vT32_{hp}")
                    for hp in range(3)]
            scn = scp.tile([128, HPC, 16], F32, tag="scn")
            score16 = scp.tile([16, HPC, 128], F32, tag="score16")

            kselb = [selp.tile([128, KEEP], BF16, tag=f"kselb{hp}", name=f"kselb{hp}")
                     for hp in range(3)]
            vselb = [selp.tile([128, KEEP], BF16, tag=f"vselb{hp}", name=f"vselb{hp}")
                     for hp in range(3)]
            vsel = [selp.tile([128, KB, HD + 1], BF16, tag=f"vsel{h}", name=f"vsel{h}")
                    for h in range(HPC)]
            for h in range(HPC):
                nc.vector.memset(vsel[h][:, :, HD:HD + 1], 1.0)

            # ---- phase 1A: q projection (Dekker split: exact enough for
            # selection) + token-major scores. All-q-first so the threshold
            # search can start while the k/v matmuls still run.
            def q_group(nb, mb, ps):
                csl = slice(mb * 128, (mb + 1) * 128)
                terms = [(wqht, xct), (wqht, xlt), (wqlt, xct)]
                for t, (w, x) in enumerate(terms):
                    for kb in range(CB):
                        nc.tensor.matmul(
                            ps, w[:, kb, csl], x[nb][:, kb, :],
                            start=(t == 0 and kb == 0),
                            stop=(t == 2 and kb == CB - 1))
                nc.vector.tensor_copy(qkT[mb][:, nb * 512:(nb + 1) * 512], ps)

            for nb in range(QC):
                sq_c = [sqp.tile([128, 512], F32, tag=f"sq{m}", name=f"sq{m}")
                        for m in range(3)]
                strip = pstrip.tile([128, 1024], F32, tag="strip", name="psq01")
                strip2 = pstrip.tile([128, 1024], F32, tag="strip", name="psq2")
                for mb in range(3):
                    ps = (strip[:, 0:512], strip[:, 512:1024], strip2[:, 0:512])[mb]
                    q_group(nb, mb, ps)
                    nc.scalar.activation(
                        sq_c[mb], ps, mybir.ActivationFunctionType.Square)
                # token-major scores per 128-token block
                for j in range(4):
                    tb = nb * 4 + j
                    ps_n = ppo.tile([128, 512], F32, tag="po", name="psn")
                    for m in range(3):
                        nc.tensor.matmul(
                            ps_n[:, 0:HPC], sq_c[m][:, j * 128:(j + 1) * 128], selm[m],
                            start=(m == 0), stop=(m == 2))
                    nc.vector.tensor_copy(scn[:, :, tb], ps_n[:, 0:HPC])

            # exact fp32 transposes: score16[tb, h, p] = scn[p, h, tb]
            # (0/1-weighted fp32 matmuls are exact)
            for h2 in range(0, HPC, 4):
                ps_t = ppo.tile([128, 512], F32, tag="po", name="pst")
                for hh in range(h2, min(h2 + 4, HPC)):
                    nc.tensor.matmul(
                        ps_t[0:16, (hh - h2) * 128:(hh - h2 + 1) * 128],
                        scn[:, hh, :], i128, start=True, stop=True)
                for hh in range(h2, min(h2 + 4, HPC)):
                    nc.vector.tensor_copy(
                        score16[:, hh, :],
                        ps_t[0:16, (hh - h2) * 128:(hh - h2 + 1) * 128])

            # ---- phase 2: 6-level 16-ary threshold search over scn
            # [128, 6, 16], interleaved with the k/v matmuls on PE.
            lo6 = bis.tile([1, HPC], F32, tag="lo6")
            nc.vector.memset(lo6, 0.0)
            thr16 = bis.tile([1, HPC, 16], F32, tag="thr16")
            c4 = bis.tile([128, HPC, 16, 16], BF16, tag="c4")
            rc = bis.tile([128, HPC * 16], F32, tag="rc")
            sel16 = bis.tile([1, HPC, 16], F32, tag="sel16")

            def next_candidates(step):
                nc.vector.scalar_tensor_tensor(
                    out=thr16,
                    in0=iotaf.unsqueeze(1).to_broadcast([1, HPC, 16]),
                    scalar=step,
                    in1=lo6.unsqueeze(-1).to_broadcast([1, HPC, 16]),
                    op0=mybir.AluOpType.mult, op1=mybir.AluOpType.add)

            next_candidates(BISECT_HI / 16)

            def search_level_pe1(thrb):
                nc.tensor.matmul(
                    thrb[:, 0:HPC * 16], ones_row,
                    thr16.rearrange("p h g -> p (h g)"), start=True, stop=True)

            def search_level_dve(thrb):
                nc.vector.tensor_tensor(
                    c4,
                    scn.unsqueeze(2).to_broadcast([128, HPC, 16, 16]),
                    thrb[:, 0:HPC * 16].rearrange("p (h g) -> p h g", h=HPC)
                        .unsqueeze(-1).to_broadcast([128, HPC, 16, 16]),
                    op=mybir.AluOpType.is_ge)
                nc.vector.tensor_reduce(
                    rc.rearrange("p (h g) -> p h g", h=HPC), c4,
                    axis=mybir.AxisListType.X, op=mybir.AluOpType.add)

            def search_level_pe2(cnt_ps):
                nc.tensor.matmul(
                    cnt_ps[0:1, 0:HPC * 16], ones_sb, rc, start=True, stop=True)

            def search_level_fin(cnt_ps, step):
                nc.vector.tensor_scalar(
                    sel16, cnt_ps[0:1, 0:HPC * 16].rearrange("p (h g) -> p h g", h=HPC),
                    float(KEEP), None, op0=mybir.AluOpType.is_ge)
                nc.vector.tensor_tensor(
                    sel16, sel16, thr16, op=mybir.AluOpType.mult)
                nc.vector.tensor_reduce(
                    lo6, sel16, axis=mybir.AxisListType.X, op=mybir.AluOpType.max)
                if step is not None:
                    next_candidates(step)

            # ---- phase 1B: k and v projections (both transposed, fp32 SBUF
            # for the gather), woven around the 6 serial search levels.
            # k first: its gathers gate the attention stream.
            def kv_item(w, dst, hp, nb):
                csl = slice(hp * 128, (hp + 1) * 128)
                ps = ppo.tile([128, 512], F32, tag="po", name="pskv")
                for kb in range(CB):
                    nc.tensor.matmul(
                        ps, w[:, kb, csl], xct[nb][:, kb, :],
                        start=(kb == 0), stop=(kb == CB - 1))
                # PSUM->SBUF on ACT (idle here), keeping the DVE queue free
                # for the bisection it is interleaved with
                nc.scalar.activation(
                    dst[:, nb * 512:(nb + 1) * 512], ps,
                    mybir.ActivationFunctionType.Copy)

            fillers = []
            for hp in range(3):
                for nb in range(QC):
                    fillers.append(lambda hp=hp, nb=nb: kv_item(wkt, kT32[hp], hp, nb))
            for hp in range(3):
                for nb in range(QC):
                    fillers.append(lambda hp=hp, nb=nb: kv_item(wvt, vT32[hp], hp, nb))
            fidx = 0

            def fill(n):
                nonlocal fidx
                for _ in range(n):
                    if fidx < len(fillers):
                        fillers[fidx]()
                        fidx += 1

            LEVELS = 6
            step = BISECT_HI / 16
            fill(2)
            for lv in range(LEVELS):
                thrb = ppo.tile([128, 512], F32, tag="po", name="thrb")
                search_level_pe1(thrb)
                fill(1)
                search_level_dve(thrb)
                cnt_ps = ppo.tile([128, 512], F32, tag="po", name="cntps")
                search_level_pe2(cnt_ps)
                fill(2)
                step = step / 16.0
                search_level_fin(cnt_ps, step if lv < LEVELS - 1 else None)

            # ---- phase 3: compaction. thr broadcast goes into the PE queue
            # right after the last search level; leftover k/v work flushes
            # behind it.
            thr128 = ppo.tile([128, 512], F32, tag="po", name="thr128")
            nc.tensor.matmul(thr128[:, 0:HPC], ones_row, lo6, start=True, stop=True)
            fill(len(fillers))  # leftover k/v work (runs behind thr128)

            # payload[p, h, f] = token idx if score>=thr else -1  (f32)
            m16 = scp.tile([16, HPC, 128], F32, tag="m16")
            nc.vector.tensor_tensor(
                m16, score16,
                thr128[0:16, 0:HPC].unsqueeze(-1).to_broadcast([16, HPC, 128]),
                op=mybir.AluOpType.is_ge)
            payload = scp.tile([16, HPC, 128], F32, tag="payload")
            nc.vector.tensor_tensor(
                payload, m16,
                valp1.unsqueeze(1).to_broadcast([16, HPC, 128]),
                op=mybir.AluOpType.mult)
            nc.vector.tensor_scalar(
                payload, payload, 1.0, None, op0=mybir.AluOpType.subtract)

            # sparse_gather per head: first 1024 selected token indices in
            # wrap-16 order. Output sized [16,128] so a tie-overrun (count >
            # 1024) cannot scribble past the tile; only [:, :64] is used.
            idxf = scp.tile([16, HPC, 128], F32, tag="idxf")
            nfound = sml.tile([1, HPC], U32, tag="nfound")
            for h in range(HPC):
                nc.gpsimd.sparse_gather(
                    idxf[:, h, :], payload[:, h, :],
                    num_found=nfound[0:1, h:h + 1])
            # clamp the -1 padding (none expected: count >= 1024 by the
            # bisection invariant) and keep the first 64 free cols
            idxc = scp.tile([16, HPC, 64], F32, tag="idxc")
            nc.vector.tensor_scalar(
                idxc, idxf[:, :, 0:64], 0.0, None, op0=mybir.AluOpType.max)
            # broadcast p -> p%16 via one exact fp32 matmul
            ps_b = ppo.tile([128, 512], F32, tag="po", name="psb")
            nc.tensor.matmul(
                ps_b[:, 0:HPC * 64], j16,
                idxc.rearrange("p h g -> p (h g)"), start=True, stop=True)
            idxb = scp.tile([128, HPC, 64], I16, tag="idxb")
            nc.vector.tensor_copy(
                idxb, ps_b[:, 0:HPC * 64].rearrange("p (h g) -> p h g", h=HPC))
            # per-pair idx layout: partitions 0-63 = even head, 64-127 = odd
            idxp = scp.tile([128, 3, 64], I16, tag="idxp")
            for hp in range(3):
                nc.vector.tensor_copy(idxp[0:64, hp, :], idxb[0:64, 2 * hp, :])
                nc.vector.tensor_copy(idxp[64:128, hp, :], idxb[64:128, 2 * hp + 1, :])

            # ---- phase 4: gathers (gpsimd) + bf16 casts + v transposes.
            # Order k0,v0,k1,k2,v1,v2: k0 unblocks the attention stream,
            # v0 the first PV pops, later pairs hide behind the stream.
            def gather_pair(src, dstb, hp):
                g32 = g32p.tile([128, KEEP], F32, tag="g32", name=f"g32_{hp}")
                nc.gpsimd.ap_gather(
                    g32, src[hp], idxp[:, hp, :],
                    channels=128, num_elems=N, d=1, num_idxs=KEEP)
                nc.vector.tensor_copy(dstb[hp], g32)

            def v_transpose(h):
                # PSUM borrows a strip tile (ppo would deadlock: its oldest
                # buffer's reader may not be emitted yet at the weave points)
                hp, j = h // 2, h % 2
                ps_v = pstrip.tile([128, 1024], F32, tag="strip", name="psvt")
                for kb in range(KB):
                    nc.tensor.matmul(
                        ps_v[:, kb * HD:(kb + 1) * HD],
                        vselb[hp][64 * j:64 * j + 64, kb * 128:(kb + 1) * 128],
                        identb[64 * j:64 * j + 64, :], start=True, stop=True)
                nc.vector.tensor_copy(vsel[h][:, :, 0:HD], ps_v[:, 0:KB * HD])

            gather_pair(kT32, kselb, 0)
            gather_pair(vT32, vselb, 0)

            # ---- phase 5: attention over gathered keys + projection.
            outT = [otp.tile([128, N], BF16, tag=f"outT{i}", name=f"outT{i}")
                    for i in range(3)]
            wp = [wpt[:, i, :] for i in range(3)]

            def proj_qb(qb):
                ps1 = ppo.tile([128, 512], F32, tag="po", name="psy1")
                ps2 = ppo.tile([128, 512], F32, tag="po", name="psy2")
                for i in range(3):
                    lhsT = outT[i][:, qb * 128:(qb + 1) * 128]
                    nc.tensor.matmul(ps1, lhsT, wp[i][:, 0:512],
                                     start=(i == 0), stop=(i == 2))
                    nc.tensor.matmul(ps2[:, 0:256], lhsT, wp[i][:, 512:768],
                                     start=(i == 0), stop=(i == 2))
                yt = yp.tile([128, C], BF16, tag="y", name="yt")
                nc.vector.tensor_copy(yt[:, 0:512], ps1)
                nc.vector.tensor_copy(yt[:, 512:768], ps2[:, 0:256])
                nc.gpsimd.dma_start(out=y_d[qb * 128:(qb + 1) * 128, :], in_=yt)

            def normalize(qc, hp, po_):
                qsl = slice(qc * 512, (qc + 1) * 512)
                for j in range(2):
                    den = sml.tile([1, 512], F32, tag="den", name="den", bufs=2)
                    nc.vector.tensor_copy(den, po_[j][HD:HD + 1, :])
                    recip = sml.tile([1, 512], F32, tag="recip", name="recip", bufs=2)
                    nc.vector.reciprocal_approx_fast(out=recip, in_=den)
                    rep = sml.tile([HD, 512], F32, tag="rep", name="rep", bufs=2)
                    nc.gpsimd.partition_broadcast(rep, recip)
                    nc.vector.tensor_mul(
                        outT[hp][64 * j:64 * j + 64, qsl], po_[j][0:HD, :], rep)

            units = [(qc, hp, tb)
                     for qc in range(QC) for hp in range(3) for tb in range(KB)]
            pending_proj = []
            po_cur = {}
            pipe = []

            def pop_unit():
                (pqc, php, ptb), ppt = pipe.pop(0)
                po_ = po_cur[(pqc, php)]
                for j in range(2):
                    nc.tensor.matmul(
                        po_[j][0:HD + 1, :], vsel[2 * php + j][:, ptb, :],
                        ppt[:, j * 512:(j + 1) * 512],
                        start=(ptb == 0), stop=(ptb == KB - 1))
                if ptb == KB - 1:
                    normalize(pqc, php, po_)
                    if php == 2:
                        pending_proj.extend(range(pqc * 4, pqc * 4 + 4))

            for ui, (qc, hp, tb) in enumerate(units):
                if tb == 0:
                    po_cur[(qc, hp)] = [
                        ppo.tile([128, 512], F32, tag="po", name="po")
                        for _ in range(2)]
                kT, qT = kselb[hp], qkT[hp]
                qsl = slice(qc * 512, (qc + 1) * 512)
                strip = pstrip.tile([128, 1024], F32, tag="strip", name="psS")
                for j in range(2):
                    nc.tensor.matmul(
                        strip[:, j * 512:(j + 1) * 512],
                        kT[64 * j:64 * j + 64, tb * 128:(tb + 1) * 128],
                        qT[64 * j:64 * j + 64, qsl], start=True, stop=True)
                pt = ptp.tile([128, 1024], BF16, tag="pt", name="pt")
                nc.scalar.activation(
                    pt, strip, mybir.ActivationFunctionType.Exp, scale=SCALE)
                pipe.append(((qc, hp, tb), pt))
                # weave gathers/transposes/remaining-gathers into the early
                # stream so each pair is ready just before it is needed
                if ui == 1:
                    v_transpose(0)
                    v_transpose(1)
                    gather_pair(kT32, kselb, 1)
                elif ui == 6:
                    gather_pair(vT32, vselb, 1)
                elif ui == 9:
                    v_transpose(2)
                    v_transpose(3)
                    gather_pair(kT32, kselb, 2)
                elif ui == 14:
                    gather_pair(vT32, vselb, 2)
                elif ui == 17:
                    v_transpose(4)
                    v_transpose(5)
                elif ui == 19:
                    nc.gpsimd.dma_start(out=thr_d[:, :], in_=lo6)
                    nc.gpsimd.dma_start(
                        out=sc_d[:, :], in_=scn.rearrange("p a b -> p (a b)"))
                lag = 4 if ui < 16 else 2
                while len(pipe) > lag:
                    pop_unit()
                if pending_proj and ui % 3 == 2:
                    proj_qb(pending_proj.pop(0))
            while pipe:
                pop_unit()
            for qb in pending_proj:
                proj_qb(qb)

    nc.compile()
    return nc


def _get_nc():
    if "nc" not in _CACHE:
        _CACHE["nc"] = _build()
    return _CACHE["nc"]


def kernel(x, w_qkv, w_proj, b_proj):
    x = np.asarray(x, dtype=np.float32)
    w_qkv = np.asarray(w_qkv, dtype=np.float32)
    w_proj = np.asarray(w_proj, dtype=np.float32)
    b_proj = np.asarray(b_proj, dtype=np.float32)

    import ml_dtypes
    bf16 = ml_dtypes.bfloat16

    selmask = np.zeros((HPC * HD, HPC), dtype=np.float32)
    for h in range(HPC):
        selmask[h * HD:(h + 1) * HD, h] = 1.0

    consts = np.zeros((128, 256), dtype=np.float32)
    consts[0:128, 0:128] = np.eye(128, dtype=np.float32)
    for p in range(16):
        consts[p, 128 + p:256:16] = 1.0
    identb = np.vstack([np.eye(HD, dtype=np.float32)] * 2).astype(bf16)

    in_maps = []
    for core in range(8):
        b, g = core // 2, core % 2
        cols = slice(g * HPC * HD, (g + 1) * HPC * HD)
        wq = np.ascontiguousarray(w_qkv[:, 0:C][:, cols])
        wqh = wq.astype(bf16)
        wql = (wq - wqh.astype(np.float32)).astype(bf16)
        xT = np.ascontiguousarray(x[b].T)
        xh = xT.astype(bf16)
        xl = (xT - xh.astype(np.float32)).astype(bf16)
        in_maps.append({
            "xh": xh,
            "xl": xl,
            "wqh": wqh,
            "wql": wql,
            "wk": np.ascontiguousarray(w_qkv[:, C:2 * C][:, cols]).astype(bf16),
            "wv": np.ascontiguousarray(w_qkv[:, 2 * C:3 * C][:, cols]).astype(bf16),
            "wp": np.ascontiguousarray(w_proj[cols, :]).astype(bf16),
            "selmask": selmask,
            "consts": consts,
            "identb": identb,
        })

    nc = _get_nc()
    r = run_bass_kernel_spmd(nc, in_maps, list(range(8)), trace=TRACE)
    LAST["exec_time_ns"] = r.exec_time_ns
    LAST["mean_exec_time_ns"] = r.mean_exec_time_ns
    LAST["results"] = r.results
    LAST["insts"] = r.instructions_and_trace
    y = np.empty((B, N, C), dtype=np.float32)
    for b in range(B):
        y[b] = (r.results[2 * b]["y"].astype(np.float32)
                + r.results[2 * b + 1]["y"].astype(np.float32))
    y = np.clip(y + b_proj, -10.0, 10.0)
    return y


# revision 65
# speedup vs baseline: 1.3440x; 1.3440x over previous
"""BiFormer sparse attention on 8 Trainium2 NeuronCores.

Problem (hardcoded): B=4, N=2048, C=768, H=12, hd=64, keep=N/2=1024.
    qkv = x @ w_qkv -> q,k,v per (B,H)
    top-1024 tokens per (B,H) by ||q|| -> gather k,v
    out = softmax(clip(q @ k_sel^T * hd^-0.5, +-50)) @ v_sel
    y = clip(out @ w_proj + b_proj, +-10)

Sharding: 8 cores = 4 batches x 2 head-groups (6 heads each). Weights are
column/row-split per head-group; the two cores of a batch produce partial
projection outputs that the host sums (+bias, clip).

Device algorithm (per core), matmuls bf16 unless noted (fp32 PSUM):
  1. qT [384,2048] via Dekker 3-term bf16 split (selection needs fp32-grade
     scores; xh/xl ship pre-split from the host). kT/vT [384,2048] single
     bf16 pass, kept in fp32 SBUF for the gather. Squared q PSUM ->
     scn [128,6,16] token-major scores + score16 [16,6,128] (exact fp32
     PE transpose) for the compaction.
  2. Per-head top-1024 threshold: 6-level 16-ary bisection on the vector
     engine over scn, overlapped with the k/v matmuls on PE.
  3. Compaction instead of masking: payload = token_idx where score>=thr
     else -1 -> gpsimd sparse_gather -> 1024 indices per head ->
     PE-broadcast to all partitions. k/v are computed token-major, spilled
     to HBM scratch as [k_e|v_e|v_o|k_o] per pair, and pulled back per
     head with dma_gather transpose=False (256B elements on the DMA
     rings; gpsimd ap_gather ucode costs ~28us/call and transpose=True
     wedges the device). The gather lands keys-major [128 keys, 8 blk,
     128]: the v half is copied to v_sel (ones column appended for the
     softmax denominator); the k half is PE-transposed to channel-major
     (odd head's k sits in lhsT cols 64-127 so its transpose lands on
     PSUM rows 64-127; the even head's M=64 pass overwrites rows 0-63).
  4. Attention over the 1024 gathered keys only (half the S/exp/PV work of
     mask-based attention): S^T = k_sel(block)^T @ q^T, two heads per
     2-bank PSUM strip, one Exp per strip; out^T accumulates
     v_sel^T @ P over 8 key blocks; row 64 = denom.
  5. Normalize by reciprocal(denom), project with row-split w_proj
     interleaved into the attention stream; y ships bf16 (host sums the
     two partials in fp32, adds bias, clips).
"""
import os
import sys

sys.path.insert(0, "/opt/trn_rl_repo")

import numpy as np

import concourse.bass as bass
import concourse.mybir as mybir
from concourse import bacc
from concourse.tile import TileContext
from concourse.bass_utils import run_bass_kernel_spmd

B, N, C, H, HD = 4, 2048, 768, 12, 64
HPC = 6                  # heads per core
KEEP = N // 2            # 1024
KB = KEEP // 128         # 8 selected-key blocks
QC = N // 512            # 4 query chunks
CB = C // 128            # 6 contraction blocks
SCALE = HD ** -0.5       # 0.125
BISECT_HI = 512.0        # scores are chi2(64)-like, max ~150 << 512
BISECT_ITERS = 24        # kept for test.py compat
F32 = mybir.dt.float32
BF16 = mybir.dt.bfloat16
I16 = mybir.dt.int16
I32 = mybir.dt.int32
U32 = mybir.dt.uint32

_CACHE = {}
TRACE = False       # set True (e.g. from test.py) to capture an NTFF profile
LAST = {}           # exec_time_ns / profile info from the most recent run


def _build():
    nc = bacc.Bacc(None, target_bir_lowering=False)
    xh_d = nc.declare_dram_parameter("xh", [C, N], BF16, isOutput=False)
    xl_d = nc.declare_dram_parameter("xl", [C, N], BF16, isOutput=False)
    wqh_d = nc.declare_dram_parameter("wqh", [C, HPC * HD], BF16, isOutput=False)
    wql_d = nc.declare_dram_parameter("wql", [C, HPC * HD], BF16, isOutput=False)
    wk_d = nc.declare_dram_parameter("wk", [C, HPC * HD], BF16, isOutput=False)
    wv_d = nc.declare_dram_parameter("wv", [C, HPC * HD], BF16, isOutput=False)
    wp_d = nc.declare_dram_parameter("wp", [HPC * HD, C], BF16, isOutput=False)
    sel_d = nc.declare_dram_parameter("selmask", [HPC * HD, HPC], F32, isOutput=False)
    # consts [128, 256] f32 = I128 | J16 (J16[p, f] = 1 if f % 16 == p,
    # rows 16.. zero); identb = I128 bf16 (for PE transposes)
    con_d = nc.declare_dram_parameter("consts", [128, 256], F32, isOutput=False)
    idb_d = nc.declare_dram_parameter("identb", [128, 128], BF16, isOutput=False)
    y_d = nc.declare_dram_parameter("y", [N, C], BF16, isOutput=True)
    thr_d = nc.declare_dram_parameter("dbg_thr", [1, HPC], F32, isOutput=True)
    sc_d = nc.declare_dram_parameter("dbg_scores", [128, HPC * 16], F32, isOutput=True)

    with TileContext(nc) as tc:
        with (
            tc.tile_pool(name="wts", bufs=1) as wts,
            tc.tile_pool(name="xc", bufs=1) as xcp,
            tc.tile_pool(name="xl", bufs=2) as xlp,
            tc.tile_pool(name="qk", bufs=1) as qkp,
            tc.tile_pool(name="kvtok", bufs=1) as kvp,
            tc.tile_pool(name="hbm", bufs=1, space="DRAM") as hbmp,
            tc.tile_pool(name="sq", bufs=1) as sqp,
            tc.tile_pool(name="sel", bufs=1) as selp,
            tc.tile_pool(name="sc", bufs=1) as scp,
            tc.tile_pool(name="small", bufs=1) as sml,
            tc.tile_pool(name="bis", bufs=1) as bis,
            tc.tile_pool(name="pt", bufs=8) as ptp,
            tc.tile_pool(name="outt", bufs=1) as otp,
            tc.tile_pool(name="y", bufs=2) as yp,
            tc.tile_pool(name="strip", bufs=2, space="PSUM") as pstrip,
            tc.tile_pool(name="po", bufs=4, space="PSUM") as ppo,
        ):
            # ---- batched loads; everything ships pre-cast bf16 (halves HBM
            # vs f32). DMA issue costs ~650ns each, so q-critical-path first.
            wqht = wts.tile([128, CB, HPC * HD], BF16, tag="wqh", name="wqht")
            nc.gpsimd.dma_start(out=wqht, in_=wqh_d.rearrange("(k p) m -> p k m", p=128))
            wqlt = wts.tile([128, CB, HPC * HD], BF16, tag="wql", name="wqlt")
            nc.gpsimd.dma_start(out=wqlt, in_=wql_d.rearrange("(k p) m -> p k m", p=128))
            xct, xlt = [], []
            for nb2 in range(QC):
                th = xcp.tile([128, CB, 512], BF16, tag=f"xc{nb2}", name=f"xc{nb2}")
                nc.gpsimd.dma_start(
                    out=th,
                    in_=xh_d[:, nb2 * 512:(nb2 + 1) * 512].rearrange("(k p) n -> p k n", p=128))
                xct.append(th)
                # xl is only read by the q Dekker terms of its own chunk:
                # 2 rotating bufs. Chunks 2/3 are DMA'd after the weights so
                # their buffer-release waits don't block the weight loads on
                # the gpsimd queue.
                xlt.append(xlp.tile([128, CB, 512], BF16, tag="xl", name=f"xl{nb2}"))
                if nb2 < 2:
                    nc.gpsimd.dma_start(
                        out=xlt[nb2],
                        in_=xl_d[:, nb2 * 512:(nb2 + 1) * 512].rearrange("(k p) n -> p k n", p=128))
            selmt = sml.tile([128, 3, HPC], F32, tag="selm", name="selmt")
            nc.gpsimd.dma_start(out=selmt, in_=sel_d.rearrange("(k p) m -> p k m", p=128))
            cont = wts.tile([128, 256], F32, tag="consts", name="cont")
            nc.gpsimd.dma_start(out=cont, in_=con_d[:, :])
            identb = wts.tile([128, 128], BF16, tag="identb", name="identb")
            nc.gpsimd.dma_start(out=identb, in_=idb_d[:, :])
            wkt = wts.tile([128, CB, HPC * HD], BF16, tag="wk", name="wkt")
            nc.gpsimd.dma_start(out=wkt, in_=wk_d.rearrange("(k p) m -> p k m", p=128))
            wvt = wts.tile([128, CB, HPC * HD], BF16, tag="wv", name="wvt")
            nc.gpsimd.dma_start(out=wvt, in_=wv_d.rearrange("(k p) m -> p k m", p=128))
            wpt = wts.tile([128, 3, C], BF16, tag="wp", name="wpt")
            nc.gpsimd.dma_start(out=wpt, in_=wp_d.rearrange("(k p) m -> p k m", p=128))
            for nb2 in range(2, QC):
                nc.gpsimd.dma_start(
                    out=xlt[nb2],
                    in_=xl_d[:, nb2 * 512:(nb2 + 1) * 512].rearrange("(k p) n -> p k n", p=128))

            i128 = cont[:, 0:128]
            j16 = cont[0:16, 128:256]
            selm = [selmt[:, i, :] for i in range(3)]

            ones_row = sml.tile([1, 128], F32, tag="ones_row")
            nc.vector.memset(ones_row, 1.0)
            ones_b = sml.tile([1, HD], BF16, tag="ones_b")
            nc.vector.memset(ones_b, 1.0)
            ones_sb = sml.tile([128, 1], F32, tag="ones_sb")
            nc.vector.memset(ones_sb, 1.0)
            iotai = sml.tile([1, 16], I32, tag="iotai")
            nc.gpsimd.iota(iotai, pattern=[[1, 16]], channel_multiplier=0)
            iotaf = sml.tile([1, 16], F32, tag="iotaf")
            nc.vector.tensor_copy(iotaf, iotai)
            # valp1[p, f] = p*128 + f + 1  (token index + 1, wrap-16 layout
            # matches the score16 transpose: token = p*128 + f)
            valp1i = sml.tile([16, 128], I32, tag="valp1i")
            nc.gpsimd.iota(valp1i, pattern=[[1, 128]], base=1, channel_multiplier=128)
            valp1 = sml.tile([16, 128], F32, tag="valp1")
            nc.vector.tensor_copy(valp1, valp1i)

            qkT = [qkp.tile([128, N], BF16, tag=f"qT{mb}", name=f"qT{mb}")
                   for mb in range(3)]
            # token-major k/v staging (SBUF then HBM scratch for the gather)
            # free layout per pair hp, 256 cols at 256*hp:
            #   [k_even | v_even | v_odd | k_odd]  (64 each)
            kvtok = kvp.tile([128, 16, 2 * HPC * HD], BF16, tag="kvtok", name="kvtok")
            kvtok_h = hbmp.tile([N, 2 * HPC * HD], BF16, tag="kvtok_h", name="kvtok_h")
            scn = scp.tile([128, HPC, 16], F32, tag="scn")
            score16 = scp.tile([16, HPC, 128], F32, tag="score16")

            # per-head gather staging (keys-major), channel-major k pair
            # tiles, and key-major v with the ones column
            kvsel = [selp.tile([128, KB, 128], BF16, tag=f"kvsel{h}", name=f"kvsel{h}")
                     for h in range(HPC)]
            kselb = [selp.tile([128, KEEP], BF16, tag=f"kselb{hp}", name=f"kselb{hp}")
                     for hp in range(3)]
            vsel = [selp.tile([128, KB, HD + 1], BF16, tag=f"vsel{h}", name=f"vsel{h}")
                    for h in range(HPC)]
            for h in range(HPC):
                nc.vector.memset(vsel[h][:, :, HD:HD + 1], 1.0)

            # ---- phase 1A: q projection (Dekker split: exact enough for
            # selection) + token-major scores. All-q-first so the threshold
            # search can start while the k/v matmuls still run.
            def q_group(nb, mb, ps):
                csl = slice(mb * 128, (mb + 1) * 128)
                terms = [(wqht, xct), (wqht, xlt), (wqlt, xct)]
                for t, (w, x) in enumerate(terms):
                    for kb in range(CB):
                        nc.tensor.matmul(
                            ps, w[:, kb, csl], x[nb][:, kb, :],
                            start=(t == 0 and kb == 0),
                            stop=(t == 2 and kb == CB - 1))
                nc.vector.tensor_copy(qkT[mb][:, nb * 512:(nb + 1) * 512], ps)

            for nb in range(QC):
                sq_c = [sqp.tile([128, 512], F32, tag=f"sq{m}", name=f"sq{m}", bufs=2)
                        for m in range(3)]
                strip = pstrip.tile([128, 1024], F32, tag="strip", name="psq01")
                strip2 = pstrip.tile([128, 1024], F32, tag="strip", name="psq2")
                for mb in range(3):
                    ps = (strip[:, 0:512], strip[:, 512:1024], strip2[:, 0:512])[mb]
                    q_group(nb, mb, ps)
                    nc.scalar.activation(
                        sq_c[mb], ps, mybir.ActivationFunctionType.Square)
                # token-major scores per 128-token block
                for j in range(4):
                    tb = nb * 4 + j
                    ps_n = ppo.tile([128, 512], F32, tag="po", name="psn")
                    for m in range(3):
                        nc.tensor.matmul(
                            ps_n[:, 0:HPC], sq_c[m][:, j * 128:(j + 1) * 128], selm[m],
                            start=(m == 0), stop=(m == 2))
                    nc.vector.tensor_copy(scn[:, :, tb], ps_n[:, 0:HPC])

            # exact fp32 transposes: score16[tb, h, p] = scn[p, h, tb]
            # (0/1-weighted fp32 matmuls are exact)
            for h2 in range(0, HPC, 4):
                ps_t = ppo.tile([128, 512], F32, tag="po", name="pst")
                for hh in range(h2, min(h2 + 4, HPC)):
                    nc.tensor.matmul(
                        ps_t[0:16, (hh - h2) * 128:(hh - h2 + 1) * 128],
                        scn[:, hh, :], i128, start=True, stop=True)
                for hh in range(h2, min(h2 + 4, HPC)):
                    nc.vector.tensor_copy(
                        score16[:, hh, :],
                        ps_t[0:16, (hh - h2) * 128:(hh - h2 + 1) * 128])

            # ---- phase 2: 6-level 16-ary threshold search over scn
            # [128, 6, 16], interleaved with the k/v matmuls on PE.
            lo6 = bis.tile([1, HPC], F32, tag="lo6")
            nc.vector.memset(lo6, 0.0)
            thr16 = bis.tile([1, HPC, 16], F32, tag="thr16")
            c4 = bis.tile([128, HPC, 16, 16], BF16, tag="c4")
            rc = bis.tile([128, HPC * 16], F32, tag="rc")
            sel16 = bis.tile([1, HPC, 16], F32, tag="sel16")

            def next_candidates(step):
                nc.vector.scalar_tensor_tensor(
                    out=thr16,
                    in0=iotaf.unsqueeze(1).to_broadcast([1, HPC, 16]),
                    scalar=step,
                    in1=lo6.unsqueeze(-1).to_broadcast([1, HPC, 16]),
                    op0=mybir.AluOpType.mult, op1=mybir.AluOpType.add)

            next_candidates(BISECT_HI / 16)

            def search_level_pe1(thrb):
                nc.tensor.matmul(
                    thrb[:, 0:HPC * 16], ones_row,
                    thr16.rearrange("p h g -> p (h g)"), start=True, stop=True)

            def search_level_dve(thrb):
                nc.vector.tensor_tensor(
                    c4,
                    scn.unsqueeze(2).to_broadcast([128, HPC, 16, 16]),
                    thrb[:, 0:HPC * 16].rearrange("p (h g) -> p h g", h=HPC)
                        .unsqueeze(-1).to_broadcast([128, HPC, 16, 16]),
                    op=mybir.AluOpType.is_ge)
                nc.vector.tensor_reduce(
                    rc.rearrange("p (h g) -> p h g", h=HPC), c4,
                    axis=mybir.AxisListType.X, op=mybir.AluOpType.add)

            def search_level_pe2(cnt_ps):
                nc.tensor.matmul(
                    cnt_ps[0:1, 0:HPC * 16], ones_sb, rc, start=True, stop=True)

            def search_level_fin(cnt_ps, step):
                nc.vector.tensor_scalar(
                    sel16, cnt_ps[0:1, 0:HPC * 16].rearrange("p (h g) -> p h g", h=HPC),
                    float(KEEP), None, op0=mybir.AluOpType.is_ge)
                nc.vector.tensor_tensor(
                    sel16, sel16, thr16, op=mybir.AluOpType.mult)
                nc.vector.tensor_reduce(
                    lo6, sel16, axis=mybir.AxisListType.X, op=mybir.AluOpType.max)
                if step is not None:
                    next_candidates(step)

            # ---- phase 1B: k and v projections (token-major, spilled to
            # HBM scratch for the dma_gather), woven around the 6 serial
            # search levels. Each filler does one 128-token block's k AND v
            # so the HBM spills (and so the gathers) aren't gated on the
            # whole v phase.
            def kv_half(tb, which):
                # one 128-token block's k OR v (half-granularity so the
                # bisection's PE round trips fit between fillers and all
                # fillers finish inside the search -> spills, so gathers,
                # start right at bisection end)
                nb, j = tb // 4, tb % 4
                w, ev, od = ((wkt, 0, 3), (wvt, 1, 2))[which]
                ps = ppo.tile([128, 512], F32, tag="po", name="pskv")
                for kb in range(CB):
                    nc.tensor.matmul(
                        ps[:, 0:HPC * HD],
                        xct[nb][:, kb, j * 128:(j + 1) * 128], w[:, kb, :],
                        start=(kb == 0), stop=(kb == CB - 1))
                # PSUM->SBUF on ACT (idle here), keeping the DVE queue free
                # for the bisection it is interleaved with. Strided copies
                # place each head's 64-col slice per the kv layout.
                row4 = kvtok[:, tb, :].rearrange("p (a b c) -> p a b c", a=3, b=4)
                src = ps[:, 0:HPC * HD].rearrange("p (a b c) -> p a b c", a=3, b=2)
                nc.scalar.activation(
                    row4[:, :, ev, :], src[:, :, 0, :],
                    mybir.ActivationFunctionType.Copy)
                nc.scalar.activation(
                    row4[:, :, od, :], src[:, :, 1, :],
                    mybir.ActivationFunctionType.Copy)
                if which == 1 and tb % 8 == 7:
                    sl = slice((tb - 7) * 128, (tb + 1) * 128)
                    nc.gpsimd.dma_start(
                        out=kvtok_h[sl, :].rearrange("(b p) c -> p b c", p=128),
                        in_=kvtok[:, tb - 7:tb + 1, :])

            fillers = []
            for tb in range(16):
                fillers.append(lambda tb=tb: kv_half(tb, 0))
                fillers.append(lambda tb=tb: kv_half(tb, 1))
            fidx = 0

            def fill(n):
                nonlocal fidx
                for _ in range(n):
                    if fidx < len(fillers):
                        fillers[fidx]()
                        fidx += 1

            LEVELS = 6
            step = BISECT_HI / 16
            fill(2)
            for lv in range(LEVELS):
                thrb = ppo.tile([128, 512], F32, tag="po", name="thrb")
                search_level_pe1(thrb)
                fill(3)
                search_level_dve(thrb)
                cnt_ps = ppo.tile([128, 512], F32, tag="po", name="cntps")
                search_level_pe2(cnt_ps)
                fill(2)
                step = step / 16.0
                search_level_fin(cnt_ps, step if lv < LEVELS - 1 else None)

            # ---- phase 3: compaction, pipelined per head pair so pair 0's
            # gathers start as early as possible. thr broadcast goes into
            # the PE queue right after the last search level; leftover k/v
            # work flushes behind it and overlaps the sparse_gathers.
            thr128 = ppo.tile([128, 512], F32, tag="po", name="thr128")
            nc.tensor.matmul(thr128[:, 0:HPC], ones_row, lo6, start=True, stop=True)

            # payload[p, h, f] = token idx if score>=thr else -1  (f32)
            m16 = scp.tile([16, HPC, 128], F32, tag="m16")
            nc.vector.tensor_tensor(
                m16, score16,
                thr128[0:16, 0:HPC].unsqueeze(-1).to_broadcast([16, HPC, 128]),
                op=mybir.AluOpType.is_ge)
            payload = scp.tile([16, HPC, 128], F32, tag="payload")
            nc.vector.tensor_tensor(
                payload, m16,
                valp1.unsqueeze(1).to_broadcast([16, HPC, 128]),
                op=mybir.AluOpType.mult)
            nc.vector.tensor_scalar(
                payload, payload, 1.0, None, op0=mybir.AluOpType.subtract)
            fill(len(fillers))  # leftover k/v work

            # per pair: sparse_gather both heads (first 1024 selected token
            # indices in wrap-16 order; output sized [16,128] so a
            # tie-overrun cannot scribble past the tile, only [:, :64]
            # used), clamp, PE-broadcast p -> p%16, int16 convert.
            idxf = scp.tile([16, HPC, 128], F32, tag="idxf")
            nfound = sml.tile([1, HPC], U32, tag="nfound")
            idxb = [scp.tile([128, 2, 64], I16, tag=f"idxb{hp}", name=f"idxb{hp}")
                    for hp in range(3)]

            def sel_pair(hp):
                for h in (2 * hp, 2 * hp + 1):
                    nc.gpsimd.sparse_gather(
                        idxf[:, h, :], payload[:, h, :],
                        num_found=nfound[0:1, h:h + 1])
                idxc = scp.tile([16, 2, 64], F32, tag=f"idxc{hp}", name=f"idxc{hp}")
                nc.vector.tensor_scalar(
                    idxc, idxf[:, 2 * hp:2 * hp + 2, 0:64], 0.0, None,
                    op0=mybir.AluOpType.max)
                ps_b = ppo.tile([128, 512], F32, tag="po", name="psb")
                nc.tensor.matmul(
                    ps_b[:, 0:128], j16,
                    idxc.rearrange("p h g -> p (h g)"), start=True, stop=True)
                nc.vector.tensor_copy(
                    idxb[hp], ps_b[:, 0:128].rearrange("p (h g) -> p h g", h=2))

            # ---- phase 4: dma_gathers (keys-major, 256B elements on the
            # DMA rings; transpose=True wedges the device, ap_gather ucode
            # costs ~28us/call) + k PE-transposes + v copies.
            def gather_kv(h):
                # even head elem = [k_e | v_e] at 256*(h//2); odd = [v_o | k_o]
                # two 512-idx halves: the Q7 descriptor prep is ~8.4ns/desc,
                # so halves cap how long a queued partition_broadcast (the
                # attention stream's normalize) can be delayed
                for half in range(2):
                    nc.gpsimd.dma_gather(
                        kvsel[h][:, 4 * half:4 * half + 4, :],
                        kvtok_h[:, 128 * h:128 * h + 128],
                        idxb[h // 2][:, h % 2, 32 * half:32 * half + 32],
                        KEEP // 2, KEEP // 2, 128,
                        elem_step=2 * HPC * HD, transpose=False)

            def v_copy(h):
                # on ACT: the DVE is both contended in this window and ~10x
                # slower than modeled for this strided copy (SBUF port
                # pressure from the concurrent gather DMA writes)
                vhalf = 0 if h % 2 else 1  # odd head's v sits in cols 0:64
                nc.scalar.activation(
                    vsel[h][:, :, 0:HD],
                    kvsel[h][:, :, vhalf * HD:(vhalf + 1) * HD],
                    mybir.ActivationFunctionType.Copy)

            def k_transpose(hp):
                # PSUM borrows a strip tile (ppo would deadlock: its oldest
                # buffer's reader may not be emitted yet at the weave points)
                ps_t = pstrip.tile([128, 1024], F32, tag="strip", name="pskt")
                for kb in range(KB):
                    # odd head first: its k sits in lhsT cols 64-127, so its
                    # transpose lands on PSUM rows 64-127 (rows 0-63 garbage
                    # from the v cols); the even head's M=64 start=True pass
                    # then overwrites rows 0-63.
                    nc.tensor.matmul(
                        ps_t[:, kb * 128:(kb + 1) * 128],
                        kvsel[2 * hp + 1][:, kb, :], identb,
                        start=True, stop=True)
                    nc.tensor.matmul(
                        ps_t[0:HD, kb * 128:(kb + 1) * 128],
                        kvsel[2 * hp][:, kb, 0:HD], identb,
                        start=True, stop=True, skip_group_check=True)
                nc.scalar.activation(
                    kselb[hp], ps_t, mybir.ActivationFunctionType.Copy)

            sel_pair(0)
            gather_kv(0)
            gather_kv(1)
            sel_pair(1)
            sel_pair(2)
            k_transpose(0)
            v_copy(0)
            v_copy(1)

            # ---- phase 5: attention over gathered keys + projection.
            outT = [otp.tile([128, N], BF16, tag=f"outT{i}", name=f"outT{i}")
                    for i in range(3)]
            wp = [wpt[:, i, :] for i in range(3)]

            def proj_qb(qb):
                ps1 = ppo.tile([128, 512], F32, tag="po", name="psy1")
                ps2 = ppo.tile([128, 512], F32, tag="po", name="psy2")
                for i in range(3):
                    lhsT = outT[i][:, qb * 128:(qb + 1) * 128]
                    nc.tensor.matmul(ps1, lhsT, wp[i][:, 0:512],
                                     start=(i == 0), stop=(i == 2))
                    nc.tensor.matmul(ps2[:, 0:256], lhsT, wp[i][:, 512:768],
                                     start=(i == 0), stop=(i == 2))
                yt = yp.tile([128, C], BF16, tag="y", name="yt")
                nc.vector.tensor_copy(yt[:, 0:512], ps1)
                nc.vector.tensor_copy(yt[:, 512:768], ps2[:, 0:256])
                nc.gpsimd.dma_start(out=y_d[qb * 128:(qb + 1) * 128, :], in_=yt)

            def normalize(qc, hp, po_):
                qsl = slice(qc * 512, (qc + 1) * 512)
                den = sml.tile([1, 1024], F32, tag="den", name="den", bufs=2)
                nc.vector.tensor_copy(den[:, 0:512], po_[0][HD:HD + 1, :])
                nc.vector.tensor_copy(den[:, 512:1024], po_[1][HD:HD + 1, :])
                recip = sml.tile([1, 1024], F32, tag="recip", name="recip", bufs=2)
                nc.vector.reciprocal_approx_fast(out=recip, in_=den)
                rep = sml.tile([HD, 1024], F32, tag="rep", name="rep", bufs=2)
                nc.gpsimd.partition_broadcast(rep, recip)
                for j in range(2):
                    nc.vector.tensor_mul(
                        outT[hp][64 * j:64 * j + 64, qsl], po_[j][0:HD, :],
                        rep[:, j * 512:(j + 1) * 512])

            units = [(qc, hp, tb)
                     for qc in range(QC) for hp in range(3) for tb in range(KB)]
            pending_proj = []
            po_cur = {}
            pipe = []

            def pop_unit():
                (pqc, php, ptb), ppt = pipe.pop(0)
                po_ = po_cur[(pqc, php)]
                for j in range(2):
                    nc.tensor.matmul(
                        po_[j][0:HD + 1, :], vsel[2 * php + j][:, ptb, :],
                        ppt[:, j * 512:(j + 1) * 512],
                        start=(ptb == 0), stop=(ptb == KB - 1))
                if ptb == KB - 1:
                    normalize(pqc, php, po_)
                    if php == 2:
                        pending_proj.extend(range(pqc * 4, pqc * 4 + 4))

            for ui, (qc, hp, tb) in enumerate(units):
                if tb == 0:
                    po_cur[(qc, hp)] = [
                        ppo.tile([128, 512], F32, tag="po", name="po")
                        for _ in range(2)]
                qT, kT = qkT[hp], kselb[hp]
                qsl = slice(qc * 512, (qc + 1) * 512)
                strip = pstrip.tile([128, 1024], F32, tag="strip", name="psS")
                for j in range(2):
                    nc.tensor.matmul(
                        strip[:, j * 512:(j + 1) * 512],
                        kT[64 * j:64 * j + 64, tb * 128:(tb + 1) * 128],
                        qT[64 * j:64 * j + 64, qsl], start=True, stop=True)
                pt = ptp.tile([128, 1024], BF16, tag="pt", name="pt")
                nc.scalar.activation(
                    pt, strip, mybir.ActivationFunctionType.Exp, scale=SCALE)
                pipe.append(((qc, hp, tb), pt))
                # weave gathers/transposes/remaining-gathers into the early
                # stream so each pair is ready just before it is needed
                if ui == 1:
                    gather_kv(2)
                    gather_kv(3)
                elif ui == 5:
                    k_transpose(1)
                    v_copy(2)
                    v_copy(3)
                elif ui == 9:
                    gather_kv(4)
                    gather_kv(5)
                elif ui == 13:
                    k_transpose(2)
                    v_copy(4)
                    v_copy(5)
                elif ui == 19:
                    nc.gpsimd.dma_start(out=thr_d[:, :], in_=lo6)
                    nc.gpsimd.dma_start(
                        out=sc_d[:, :], in_=scn.rearrange("p a b -> p (a b)"))
                lag = 4 if ui < 16 else 2
                while len(pipe) > lag:
                    pop_unit()
                if pending_proj and ui % 3 == 2:
                    proj_qb(pending_proj.pop(0))
            while pipe:
                pop_unit()
            for qb in pending_proj:
                proj_qb(qb)

    nc.compile()
    return nc


def _get_nc():
    if "nc" not in _CACHE:
        _CACHE["nc"] = _build()
    return _CACHE["nc"]


def kernel(x, w_qkv, w_proj, b_proj):
    x = np.asarray(x, dtype=np.float32)
    w_qkv = np.asarray(w_qkv, dtype=np.float32)
    w_proj = np.asarray(w_proj, dtype=np.float32)
    b_proj = np.asarray(b_proj, dtype=np.float32)

    import ml_dtypes
    bf16 = ml_dtypes.bfloat16

    selmask = np.zeros((HPC * HD, HPC), dtype=np.float32)
    for h in range(HPC):
        selmask[h * HD:(h + 1) * HD, h] = 1.0

    consts = np.zeros((128, 256), dtype=np.float32)
    consts[0:128, 0:128] = np.eye(128, dtype=np.float32)
    for p in range(16):
        consts[p, 128 + p:256:16] = 1.0
    identb = np.eye(128, dtype=np.float32).astype(bf16)

    in_maps = []
    for core in range(8):
        b, g = core // 2, core % 2
        cols = slice(g * HPC * HD, (g + 1) * HPC * HD)
        wq = np.ascontiguousarray(w_qkv[:, 0:C][:, cols])
        wqh = wq.astype(bf16)
        wql = (wq - wqh.astype(np.float32)).astype(bf16)
        xT = np.ascontiguousarray(x[b].T)
        xh = xT.astype(bf16)
        xl = (xT - xh.astype(np.float32)).astype(bf16)
        in_maps.append({
            "xh": xh,
            "xl": xl,
            "wqh": wqh,
            "wql": wql,
            "wk": np.ascontiguousarray(w_qkv[:, C:2 * C][:, cols]).astype(bf16),
            "wv": np.ascontiguousarray(w_qkv[:, 2 * C:3 * C][:, cols]).astype(bf16),
            "wp": np.ascontiguousarray(w_proj[cols, :]).astype(bf16),
            "selmask": selmask,
            "consts": consts,
            "identb": identb,
        })

    nc = _get_nc()
    r = run_bass_kernel_spmd(nc, in_maps, list(range(8)), trace=TRACE)
    LAST["exec_time_ns"] = r.exec_time_ns
    LAST["mean_exec_time_ns"] = r.mean_exec_time_ns
    LAST["results"] = r.results
    LAST["insts"] = r.instructions_and_trace
    y = np.empty((B, N, C), dtype=np.float32)
    for b in range(B):
        y[b] = (r.results[2 * b]["y"].astype(np.float32)
                + r.results[2 * b + 1]["y"].astype(np.float32))
    y = np.clip(y + b_proj, -10.0, 10.0)
    return y


# revision 67
# speedup vs baseline: 1.3666x; 1.0168x over previous
"""BiFormer sparse attention on 8 Trainium2 NeuronCores.

Problem (hardcoded): B=4, N=2048, C=768, H=12, hd=64, keep=N/2=1024.
    qkv = x @ w_qkv -> q,k,v per (B,H)
    top-1024 tokens per (B,H) by ||q|| -> gather k,v
    out = softmax(clip(q @ k_sel^T * hd^-0.5, +-50)) @ v_sel
    y = clip(out @ w_proj + b_proj, +-10)

Sharding: 8 cores = 4 batches x 2 head-groups (6 heads each). Weights are
column/row-split per head-group; the two cores of a batch produce partial
projection outputs that the host sums (+bias, clip).

Device algorithm (per core), matmuls bf16 unless noted (fp32 PSUM):
  1. qT [384,2048] via Dekker 3-term bf16 split (selection needs fp32-grade
     scores; xh/xl ship pre-split from the host). kT/vT [384,2048] single
     bf16 pass, kept in fp32 SBUF for the gather. Squared q PSUM ->
     scn [128,6,16] token-major scores + score16 [16,6,128] (exact fp32
     PE transpose) for the compaction.
  2. Per-head top-1024 threshold: 6-level 16-ary bisection on the vector
     engine over scn, overlapped with the k/v matmuls on PE.
  3. Compaction instead of masking: payload = token_idx where score>=thr
     else -1 -> gpsimd sparse_gather -> 1024 indices per head ->
     PE-broadcast to all partitions. k/v are computed token-major, spilled
     to HBM scratch as [k_e|v_e|v_o|k_o] per pair, and pulled back per
     head with dma_gather transpose=False (256B elements on the DMA
     rings; gpsimd ap_gather ucode costs ~28us/call and transpose=True
     wedges the device). The gather lands keys-major [128 keys, 8 blk,
     128]: the v half is copied to v_sel (ones column appended for the
     softmax denominator); the k half is PE-transposed to channel-major
     (odd head's k sits in lhsT cols 64-127 so its transpose lands on
     PSUM rows 64-127; the even head's M=64 pass overwrites rows 0-63).
  4. Attention over the 1024 gathered keys only (half the S/exp/PV work of
     mask-based attention): S^T = k_sel(block)^T @ q^T, two heads per
     2-bank PSUM strip, one Exp per strip; out^T accumulates
     v_sel^T @ P over 8 key blocks; row 64 = denom.
  5. Normalize by reciprocal(denom), project with row-split w_proj
     interleaved into the attention stream; y ships bf16 (host sums the
     two partials in fp32, adds bias, clips).
"""
import os
import sys

sys.path.insert(0, "/opt/trn_rl_repo")

import numpy as np

import concourse.bass as bass
import concourse.mybir as mybir
from concourse import bacc
from concourse.tile import TileContext
from concourse.bass_utils import run_bass_kernel_spmd

B, N, C, H, HD = 4, 2048, 768, 12, 64
HPC = 6                  # heads per core
KEEP = N // 2            # 1024
KB = KEEP // 128         # 8 selected-key blocks
QC = N // 512            # 4 query chunks
CB = C // 128            # 6 contraction blocks
SCALE = HD ** -0.5       # 0.125
BISECT_HI = 512.0        # scores are chi2(64)-like, max ~150 << 512
BISECT_ITERS = 24        # kept for test.py compat
F32 = mybir.dt.float32
BF16 = mybir.dt.bfloat16
I16 = mybir.dt.int16
I32 = mybir.dt.int32
U32 = mybir.dt.uint32

_CACHE = {}
TRACE = False       # set True (e.g. from test.py) to capture an NTFF profile
LAST = {}           # exec_time_ns / profile info from the most recent run


def _build():
    nc = bacc.Bacc(None, target_bir_lowering=False)
    xh_d = nc.declare_dram_parameter("xh", [C, N], BF16, isOutput=False)
    xl_d = nc.declare_dram_parameter("xl", [C, N], BF16, isOutput=False)
    wqh_d = nc.declare_dram_parameter("wqh", [C, HPC * HD], BF16, isOutput=False)
    wql_d = nc.declare_dram_parameter("wql", [C, HPC * HD], BF16, isOutput=False)
    wk_d = nc.declare_dram_parameter("wk", [C, HPC * HD], BF16, isOutput=False)
    wv_d = nc.declare_dram_parameter("wv", [C, HPC * HD], BF16, isOutput=False)
    wp_d = nc.declare_dram_parameter("wp", [HPC * HD, C], BF16, isOutput=False)
    sel_d = nc.declare_dram_parameter("selmask", [HPC * HD, HPC], F32, isOutput=False)
    # consts [128, 256] f32 = I128 | J16 (J16[p, f] = 1 if f % 16 == p,
    # rows 16.. zero); identb = I128 bf16 (for PE transposes)
    con_d = nc.declare_dram_parameter("consts", [128, 256], F32, isOutput=False)
    idb_d = nc.declare_dram_parameter("identb", [128, 128], BF16, isOutput=False)
    y_d = nc.declare_dram_parameter("y", [N, C], BF16, isOutput=True)
    thr_d = nc.declare_dram_parameter("dbg_thr", [1, HPC], F32, isOutput=True)
    sc_d = nc.declare_dram_parameter("dbg_scores", [128, HPC * 16], F32, isOutput=True)

    with TileContext(nc) as tc:
        with (
            tc.tile_pool(name="wts", bufs=1) as wts,
            tc.tile_pool(name="xc", bufs=1) as xcp,
            tc.tile_pool(name="xl", bufs=2) as xlp,
            tc.tile_pool(name="qk", bufs=1) as qkp,
            tc.tile_pool(name="kvtok", bufs=1) as kvp,
            tc.tile_pool(name="hbm", bufs=1, space="DRAM") as hbmp,
            tc.tile_pool(name="sq", bufs=1) as sqp,
            tc.tile_pool(name="sel", bufs=1) as selp,
            tc.tile_pool(name="sc", bufs=1) as scp,
            tc.tile_pool(name="small", bufs=1) as sml,
            tc.tile_pool(name="bis", bufs=1) as bis,
            tc.tile_pool(name="pt", bufs=8) as ptp,
            tc.tile_pool(name="outt", bufs=1) as otp,
            tc.tile_pool(name="y", bufs=2) as yp,
            tc.tile_pool(name="strip", bufs=2, space="PSUM") as pstrip,
            tc.tile_pool(name="po", bufs=4, space="PSUM") as ppo,
        ):
            # ---- batched loads; everything ships pre-cast bf16 (halves HBM
            # vs f32). DMA issue costs ~650ns each, so q-critical-path first.
            wqht = wts.tile([128, CB, HPC * HD], BF16, tag="wqh", name="wqht")
            nc.gpsimd.dma_start(out=wqht, in_=wqh_d.rearrange("(k p) m -> p k m", p=128))
            wqlt = wts.tile([128, CB, HPC * HD], BF16, tag="wql", name="wqlt")
            nc.gpsimd.dma_start(out=wqlt, in_=wql_d.rearrange("(k p) m -> p k m", p=128))
            xct, xlt = [], []
            for nb2 in range(QC):
                th = xcp.tile([128, CB, 512], BF16, tag=f"xc{nb2}", name=f"xc{nb2}")
                nc.gpsimd.dma_start(
                    out=th,
                    in_=xh_d[:, nb2 * 512:(nb2 + 1) * 512].rearrange("(k p) n -> p k n", p=128))
                xct.append(th)
                # xl is only read by the q Dekker terms of its own chunk:
                # 2 rotating bufs. Chunks 2/3 are DMA'd after the weights so
                # their buffer-release waits don't block the weight loads on
                # the gpsimd queue.
                xlt.append(xlp.tile([128, CB, 512], BF16, tag="xl", name=f"xl{nb2}"))
                if nb2 < 2:
                    nc.gpsimd.dma_start(
                        out=xlt[nb2],
                        in_=xl_d[:, nb2 * 512:(nb2 + 1) * 512].rearrange("(k p) n -> p k n", p=128))
            selmt = sml.tile([128, 3, HPC], F32, tag="selm", name="selmt")
            nc.gpsimd.dma_start(out=selmt, in_=sel_d.rearrange("(k p) m -> p k m", p=128))
            cont = wts.tile([128, 256], F32, tag="consts", name="cont")
            nc.gpsimd.dma_start(out=cont, in_=con_d[:, :])
            identb = wts.tile([128, 128], BF16, tag="identb", name="identb")
            nc.gpsimd.dma_start(out=identb, in_=idb_d[:, :])
            wkt = wts.tile([128, CB, HPC * HD], BF16, tag="wk", name="wkt")
            nc.gpsimd.dma_start(out=wkt, in_=wk_d.rearrange("(k p) m -> p k m", p=128))
            wvt = wts.tile([128, CB, HPC * HD], BF16, tag="wv", name="wvt")
            nc.gpsimd.dma_start(out=wvt, in_=wv_d.rearrange("(k p) m -> p k m", p=128))
            wpt = wts.tile([128, 3, C], BF16, tag="wp", name="wpt")
            nc.gpsimd.dma_start(out=wpt, in_=wp_d.rearrange("(k p) m -> p k m", p=128))
            for nb2 in range(2, QC):
                nc.gpsimd.dma_start(
                    out=xlt[nb2],
                    in_=xl_d[:, nb2 * 512:(nb2 + 1) * 512].rearrange("(k p) n -> p k n", p=128))

            i128 = cont[:, 0:128]
            j16 = cont[0:16, 128:256]
            selm = [selmt[:, i, :] for i in range(3)]

            ones_row = sml.tile([1, 128], F32, tag="ones_row")
            nc.vector.memset(ones_row, 1.0)
            ones_b = sml.tile([1, HD], BF16, tag="ones_b")
            nc.vector.memset(ones_b, 1.0)
            ones_sb = sml.tile([128, 1], F32, tag="ones_sb")
            nc.vector.memset(ones_sb, 1.0)
            iotai = sml.tile([1, 16], I32, tag="iotai")
            nc.gpsimd.iota(iotai, pattern=[[1, 16]], channel_multiplier=0)
            iotaf = sml.tile([1, 16], F32, tag="iotaf")
            nc.vector.tensor_copy(iotaf, iotai)
            # valp1[p, f] = p*128 + f + 1  (token index + 1, wrap-16 layout
            # matches the score16 transpose: token = p*128 + f)
            valp1i = sml.tile([16, 128], I32, tag="valp1i")
            nc.gpsimd.iota(valp1i, pattern=[[1, 128]], base=1, channel_multiplier=128)
            valp1 = sml.tile([16, 128], F32, tag="valp1")
            nc.vector.tensor_copy(valp1, valp1i)

            qkT = [qkp.tile([128, N], BF16, tag=f"qT{mb}", name=f"qT{mb}")
                   for mb in range(3)]
            # token-major k/v staging (SBUF then HBM scratch for the gather)
            # free layout per pair hp, 256 cols at 256*hp:
            #   [k_even | v_even | v_odd | k_odd]  (64 each)
            kvtok = kvp.tile([128, 16, 2 * HPC * HD], BF16, tag="kvtok", name="kvtok")
            kvtok_h = hbmp.tile([N, 2 * HPC * HD], BF16, tag="kvtok_h", name="kvtok_h")
            scn = scp.tile([128, HPC, 16], F32, tag="scn")
            score16 = scp.tile([16, HPC, 128], F32, tag="score16")

            # per-head gather staging (keys-major), channel-major k pair
            # tiles, and key-major v with the ones column
            kvsel = [selp.tile([128, KB, 128], BF16, tag=f"kvsel{h}", name=f"kvsel{h}")
                     for h in range(HPC)]
            kselb = [selp.tile([128, KEEP], BF16, tag=f"kselb{hp}", name=f"kselb{hp}")
                     for hp in range(3)]
            vsel = [selp.tile([128, KB, HD + 1], BF16, tag=f"vsel{h}", name=f"vsel{h}")
                    for h in range(HPC)]
            for h in range(HPC):
                nc.vector.memset(vsel[h][:, :, HD:HD + 1], 1.0)

            # ---- phase 1A: q projection (Dekker split: exact enough for
            # selection) + token-major scores. All-q-first so the threshold
            # search can start while the k/v matmuls still run.
            def q_group(nb, mb, ps):
                csl = slice(mb * 128, (mb + 1) * 128)
                terms = [(wqht, xct), (wqht, xlt), (wqlt, xct)]
                for t, (w, x) in enumerate(terms):
                    for kb in range(CB):
                        nc.tensor.matmul(
                            ps, w[:, kb, csl], x[nb][:, kb, :],
                            start=(t == 0 and kb == 0),
                            stop=(t == 2 and kb == CB - 1))
                nc.vector.tensor_copy(qkT[mb][:, nb * 512:(nb + 1) * 512], ps)

            for nb in range(QC):
                sq_c = [sqp.tile([128, 512], F32, tag=f"sq{m}", name=f"sq{m}", bufs=2)
                        for m in range(3)]
                strip = pstrip.tile([128, 1024], F32, tag="strip", name="psq01")
                strip2 = pstrip.tile([128, 1024], F32, tag="strip", name="psq2")
                for mb in range(3):
                    ps = (strip[:, 0:512], strip[:, 512:1024], strip2[:, 0:512])[mb]
                    q_group(nb, mb, ps)
                    nc.scalar.activation(
                        sq_c[mb], ps, mybir.ActivationFunctionType.Square)
                # token-major scores per 128-token block
                for j in range(4):
                    tb = nb * 4 + j
                    ps_n = ppo.tile([128, 512], F32, tag="po", name="psn")
                    for m in range(3):
                        nc.tensor.matmul(
                            ps_n[:, 0:HPC], sq_c[m][:, j * 128:(j + 1) * 128], selm[m],
                            start=(m == 0), stop=(m == 2))
                    nc.vector.tensor_copy(scn[:, :, tb], ps_n[:, 0:HPC])

            # exact fp32 transposes: score16[tb, h, p] = scn[p, h, tb]
            # (0/1-weighted fp32 matmuls are exact)
            for h2 in range(0, HPC, 4):
                ps_t = ppo.tile([128, 512], F32, tag="po", name="pst")
                for hh in range(h2, min(h2 + 4, HPC)):
                    nc.tensor.matmul(
                        ps_t[0:16, (hh - h2) * 128:(hh - h2 + 1) * 128],
                        scn[:, hh, :], i128, start=True, stop=True)
                for hh in range(h2, min(h2 + 4, HPC)):
                    nc.vector.tensor_copy(
                        score16[:, hh, :],
                        ps_t[0:16, (hh - h2) * 128:(hh - h2 + 1) * 128])

            # ---- phase 2: 6-level 16-ary threshold search over scn
            # [128, 6, 16], interleaved with the k/v matmuls on PE.
            lo6 = bis.tile([1, HPC], F32, tag="lo6")
            nc.vector.memset(lo6, 0.0)
            thr16 = bis.tile([1, HPC, 16], F32, tag="thr16")
            c4 = bis.tile([128, HPC, 16, 16], BF16, tag="c4")
            rc = bis.tile([128, HPC * 16], F32, tag="rc")
            sel16 = bis.tile([1, HPC, 16], F32, tag="sel16")

            def next_candidates(step):
                nc.vector.scalar_tensor_tensor(
                    out=thr16,
                    in0=iotaf.unsqueeze(1).to_broadcast([1, HPC, 16]),
                    scalar=step,
                    in1=lo6.unsqueeze(-1).to_broadcast([1, HPC, 16]),
                    op0=mybir.AluOpType.mult, op1=mybir.AluOpType.add)

            next_candidates(BISECT_HI / 16)

            def search_level_pe1(thrb):
                nc.tensor.matmul(
                    thrb[:, 0:HPC * 16], ones_row,
                    thr16.rearrange("p h g -> p (h g)"), start=True, stop=True)

            def search_level_dve(thrb):
                nc.vector.tensor_tensor(
                    c4,
                    scn.unsqueeze(2).to_broadcast([128, HPC, 16, 16]),
                    thrb[:, 0:HPC * 16].rearrange("p (h g) -> p h g", h=HPC)
                        .unsqueeze(-1).to_broadcast([128, HPC, 16, 16]),
                    op=mybir.AluOpType.is_ge)
                nc.vector.tensor_reduce(
                    rc.rearrange("p (h g) -> p h g", h=HPC), c4,
                    axis=mybir.AxisListType.X, op=mybir.AluOpType.add)

            def search_level_pe2(cnt_ps):
                nc.tensor.matmul(
                    cnt_ps[0:1, 0:HPC * 16], ones_sb, rc, start=True, stop=True)

            def search_level_fin(cnt_ps, step):
                nc.vector.tensor_scalar(
                    sel16, cnt_ps[0:1, 0:HPC * 16].rearrange("p (h g) -> p h g", h=HPC),
                    float(KEEP), None, op0=mybir.AluOpType.is_ge)
                nc.vector.tensor_tensor(
                    sel16, sel16, thr16, op=mybir.AluOpType.mult)
                nc.vector.tensor_reduce(
                    lo6, sel16, axis=mybir.AxisListType.X, op=mybir.AluOpType.max)
                if step is not None:
                    next_candidates(step)

            # ---- phase 1B: k and v projections (token-major, spilled to
            # HBM scratch for the dma_gather), woven around the 6 serial
            # search levels. Each filler does one 128-token block's k AND v
            # so the HBM spills (and so the gathers) aren't gated on the
            # whole v phase.
            def kv_half(tb, which):
                # one 128-token block's k OR v (half-granularity so the
                # bisection's PE round trips fit between fillers and all
                # fillers finish inside the search -> spills, so gathers,
                # start right at bisection end)
                nb, j = tb // 4, tb % 4
                w, ev, od = ((wkt, 0, 3), (wvt, 1, 2))[which]
                ps = ppo.tile([128, 512], F32, tag="po", name="pskv")
                for kb in range(CB):
                    nc.tensor.matmul(
                        ps[:, 0:HPC * HD],
                        xct[nb][:, kb, j * 128:(j + 1) * 128], w[:, kb, :],
                        start=(kb == 0), stop=(kb == CB - 1))
                # PSUM->SBUF on ACT (idle here), keeping the DVE queue free
                # for the bisection it is interleaved with. Strided copies
                # place each head's 64-col slice per the kv layout.
                row4 = kvtok[:, tb, :].rearrange("p (a b c) -> p a b c", a=3, b=4)
                src = ps[:, 0:HPC * HD].rearrange("p (a b c) -> p a b c", a=3, b=2)
                nc.scalar.activation(
                    row4[:, :, ev, :], src[:, :, 0, :],
                    mybir.ActivationFunctionType.Copy)
                nc.scalar.activation(
                    row4[:, :, od, :], src[:, :, 1, :],
                    mybir.ActivationFunctionType.Copy)
                if which == 1 and tb % 8 == 7:
                    sl = slice((tb - 7) * 128, (tb + 1) * 128)
                    nc.gpsimd.dma_start(
                        out=kvtok_h[sl, :].rearrange("(b p) c -> p b c", p=128),
                        in_=kvtok[:, tb - 7:tb + 1, :])

            fillers = []
            for tb in range(16):
                fillers.append(lambda tb=tb: kv_half(tb, 0))
                fillers.append(lambda tb=tb: kv_half(tb, 1))
            fidx = 0

            def fill(n):
                nonlocal fidx
                for _ in range(n):
                    if fidx < len(fillers):
                        fillers[fidx]()
                        fidx += 1

            LEVELS = 6
            step = BISECT_HI / 16
            fill(2)
            for lv in range(LEVELS):
                thrb = ppo.tile([128, 512], F32, tag="po", name="thrb")
                search_level_pe1(thrb)
                fill(3)
                search_level_dve(thrb)
                cnt_ps = ppo.tile([128, 512], F32, tag="po", name="cntps")
                search_level_pe2(cnt_ps)
                fill(2)
                step = step / 16.0
                search_level_fin(cnt_ps, step if lv < LEVELS - 1 else None)

            # ---- phase 3: compaction, pipelined per head pair so pair 0's
            # gathers start as early as possible. thr broadcast goes into
            # the PE queue right after the last search level; leftover k/v
            # work flushes behind it and overlaps the sparse_gathers.
            thr128 = ppo.tile([128, 512], F32, tag="po", name="thr128")
            nc.tensor.matmul(thr128[:, 0:HPC], ones_row, lo6, start=True, stop=True)

            # payload[p, h, f] = token idx if score>=thr else -1  (f32)
            m16 = scp.tile([16, HPC, 128], F32, tag="m16")
            nc.vector.tensor_tensor(
                m16, score16,
                thr128[0:16, 0:HPC].unsqueeze(-1).to_broadcast([16, HPC, 128]),
                op=mybir.AluOpType.is_ge)
            payload = scp.tile([16, HPC, 128], F32, tag="payload")
            nc.vector.tensor_tensor(
                payload, m16,
                valp1.unsqueeze(1).to_broadcast([16, HPC, 128]),
                op=mybir.AluOpType.mult)
            nc.vector.tensor_scalar(
                payload, payload, 1.0, None, op0=mybir.AluOpType.subtract)
            fill(len(fillers))  # leftover k/v work

            # per pair: sparse_gather both heads (first 1024 selected token
            # indices in wrap-16 order; output sized [16,128] so a
            # tie-overrun cannot scribble past the tile, only [:, :64]
            # used), clamp, PE-broadcast p -> p%16, int16 convert.
            idxf = scp.tile([16, HPC, 128], F32, tag="idxf")
            nfound = sml.tile([1, HPC], U32, tag="nfound")
            idxb = [scp.tile([128, 2, 64], I16, tag=f"idxb{hp}", name=f"idxb{hp}")
                    for hp in range(3)]

            def sel_pair(hp):
                for h in (2 * hp, 2 * hp + 1):
                    nc.gpsimd.sparse_gather(
                        idxf[:, h, :], payload[:, h, :],
                        num_found=nfound[0:1, h:h + 1])
                idxc = scp.tile([16, 2, 64], F32, tag=f"idxc{hp}", name=f"idxc{hp}")
                nc.vector.tensor_scalar(
                    idxc, idxf[:, 2 * hp:2 * hp + 2, 0:64], 0.0, None,
                    op0=mybir.AluOpType.max)
                ps_b = ppo.tile([128, 512], F32, tag="po", name="psb")
                nc.tensor.matmul(
                    ps_b[:, 0:128], j16,
                    idxc.rearrange("p h g -> p (h g)"), start=True, stop=True)
                nc.vector.tensor_copy(
                    idxb[hp], ps_b[:, 0:128].rearrange("p (h g) -> p h g", h=2))

            # ---- phase 4: dma_gathers (keys-major, 256B elements on the
            # DMA rings; transpose=True wedges the device, ap_gather ucode
            # costs ~28us/call) + k PE-transposes + v copies.
            def gather_kv(h):
                # even head elem = [k_e | v_e] at 256*(h//2); odd = [v_o | k_o]
                # two 512-idx halves: the Q7 descriptor prep is ~8.4ns/desc,
                # so halves cap how long a queued partition_broadcast (the
                # attention stream's normalize) can be delayed
                for half in range(2):
                    nc.gpsimd.dma_gather(
                        kvsel[h][:, 4 * half:4 * half + 4, :],
                        kvtok_h[:, 128 * h:128 * h + 128],
                        idxb[h // 2][:, h % 2, 32 * half:32 * half + 32],
                        KEEP // 2, KEEP // 2, 128,
                        elem_step=2 * HPC * HD, transpose=False)

            def v_copy(h):
                # on ACT: the DVE is both contended in this window and ~10x
                # slower than modeled for this strided copy (SBUF port
                # pressure from the concurrent gather DMA writes)
                vhalf = 0 if h % 2 else 1  # odd head's v sits in cols 0:64
                nc.scalar.activation(
                    vsel[h][:, :, 0:HD],
                    kvsel[h][:, :, vhalf * HD:(vhalf + 1) * HD],
                    mybir.ActivationFunctionType.Copy)

            def k_transpose(hp):
                # PSUM borrows a strip tile (ppo would deadlock: its oldest
                # buffer's reader may not be emitted yet at the weave points)
                ps_t = pstrip.tile([128, 1024], F32, tag="strip", name="pskt")
                for kb in range(KB):
                    # odd head first: its k sits in lhsT cols 64-127, so its
                    # transpose lands on PSUM rows 64-127 (rows 0-63 garbage
                    # from the v cols); the even head's M=64 start=True pass
                    # then overwrites rows 0-63.
                    nc.tensor.matmul(
                        ps_t[:, kb * 128:(kb + 1) * 128],
                        kvsel[2 * hp + 1][:, kb, :], identb,
                        start=True, stop=True)
                    nc.tensor.matmul(
                        ps_t[0:HD, kb * 128:(kb + 1) * 128],
                        kvsel[2 * hp][:, kb, 0:HD], identb,
                        start=True, stop=True, skip_group_check=True)
                nc.scalar.activation(
                    kselb[hp], ps_t, mybir.ActivationFunctionType.Copy)

            sel_pair(0)
            gather_kv(0)
            gather_kv(1)
            sel_pair(1)
            sel_pair(2)
            k_transpose(0)
            v_copy(0)
            v_copy(1)

            # ---- phase 5: attention over gathered keys + projection.
            outT = [otp.tile([128, N], BF16, tag=f"outT{i}", name=f"outT{i}")
                    for i in range(3)]
            wp = [wpt[:, i, :] for i in range(3)]

            def proj_qb(qb):
                ps1 = ppo.tile([128, 512], F32, tag="po", name="psy1")
                ps2 = ppo.tile([128, 512], F32, tag="po", name="psy2")
                for i in range(3):
                    lhsT = outT[i][:, qb * 128:(qb + 1) * 128]
                    nc.tensor.matmul(ps1, lhsT, wp[i][:, 0:512],
                                     start=(i == 0), stop=(i == 2))
                    nc.tensor.matmul(ps2[:, 0:256], lhsT, wp[i][:, 512:768],
                                     start=(i == 0), stop=(i == 2))
                yt = yp.tile([128, C], BF16, tag="y", name="yt")
                nc.vector.tensor_copy(yt[:, 0:512], ps1)
                nc.vector.tensor_copy(yt[:, 512:768], ps2[:, 0:256])
                nc.gpsimd.dma_start(out=y_d[qb * 128:(qb + 1) * 128, :], in_=yt)

            def normalize(qc, hp, po_):
                qsl = slice(qc * 512, (qc + 1) * 512)
                den = sml.tile([1, 1024], F32, tag="den", name="den", bufs=2)
                nc.vector.tensor_copy(den[:, 0:512], po_[0][HD:HD + 1, :])
                nc.vector.tensor_copy(den[:, 512:1024], po_[1][HD:HD + 1, :])
                recip = sml.tile([1, 1024], F32, tag="recip", name="recip", bufs=2)
                nc.vector.reciprocal_approx_fast(out=recip, in_=den)
                rep = sml.tile([HD, 1024], F32, tag="rep", name="rep", bufs=2)
                nc.gpsimd.partition_broadcast(rep, recip)
                for j in range(2):
                    nc.vector.tensor_mul(
                        outT[hp][64 * j:64 * j + 64, qsl], po_[j][0:HD, :],
                        rep[:, j * 512:(j + 1) * 512])

            units = [(qc, hp, tb)
                     for qc in range(QC) for hp in range(3) for tb in range(KB)]
            pending_proj = []
            po_cur = {}
            pipe = []

            def pop_unit():
                (pqc, php, ptb), ppt = pipe.pop(0)
                po_ = po_cur[(pqc, php)]
                for j in range(2):
                    nc.tensor.matmul(
                        po_[j][0:HD + 1, :], vsel[2 * php + j][:, ptb, :],
                        ppt[:, j * 512:(j + 1) * 512],
                        start=(ptb == 0), stop=(ptb == KB - 1))
                if ptb == KB - 1:
                    normalize(pqc, php, po_)
                    if php == 2:
                        pending_proj.extend(range(pqc * 4, pqc * 4 + 4))

            for ui, (qc, hp, tb) in enumerate(units):
                if tb == 0:
                    po_cur[(qc, hp)] = [
                        ppo.tile([128, 512], F32, tag="po", name="po")
                        for _ in range(2)]
                qT, kT = qkT[hp], kselb[hp]
                qsl = slice(qc * 512, (qc + 1) * 512)
                strip = pstrip.tile([128, 1024], F32, tag="strip", name="psS")
                for j in range(2):
                    nc.tensor.matmul(
                        strip[:, j * 512:(j + 1) * 512],
                        kT[64 * j:64 * j + 64, tb * 128:(tb + 1) * 128],
                        qT[64 * j:64 * j + 64, qsl], start=True, stop=True)
                pt = ptp.tile([128, 1024], BF16, tag="pt", name="pt")
                nc.scalar.activation(
                    pt, strip, mybir.ActivationFunctionType.Exp, scale=SCALE)
                pipe.append(((qc, hp, tb), pt))
                # weave gathers/transposes/remaining-gathers into the early
                # stream so each pair is ready just before it is needed
                if ui == 1:
                    gather_kv(2)
                    gather_kv(3)
                elif ui == 5:
                    k_transpose(1)
                    v_copy(2)
                    v_copy(3)
                elif ui == 9:
                    gather_kv(4)
                    gather_kv(5)
                elif ui == 13:
                    k_transpose(2)
                    v_copy(4)
                    v_copy(5)
                elif ui == 90:
                    # dbg outputs late: keeps the gpsimd queue clear for the
                    # gather halves the early stream is paced by
                    nc.gpsimd.dma_start(out=thr_d[:, :], in_=lo6)
                    nc.gpsimd.dma_start(
                        out=sc_d[:, :], in_=scn.rearrange("p a b -> p (a b)"))
                lag = 4 if ui < 16 else 2
                while len(pipe) > lag:
                    pop_unit()
                if pending_proj and (ui % 3 == 2 or ui >= 88):
                    proj_qb(pending_proj.pop(0))
            while pipe:
                pop_unit()
            for qb in pending_proj:
                proj_qb(qb)

    nc.compile()
    return nc


def _get_nc():
    if "nc" not in _CACHE:
        _CACHE["nc"] = _build()
    return _CACHE["nc"]


def kernel(x, w_qkv, w_proj, b_proj):
    x = np.asarray(x, dtype=np.float32)
    w_qkv = np.asarray(w_qkv, dtype=np.float32)
    w_proj = np.asarray(w_proj, dtype=np.float32)
    b_proj = np.asarray(b_proj, dtype=np.float32)

    import ml_dtypes
    bf16 = ml_dtypes.bfloat16

    selmask = np.zeros((HPC * HD, HPC), dtype=np.float32)
    for h in range(HPC):
        selmask[h * HD:(h + 1) * HD, h] = 1.0

    consts = np.zeros((128, 256), dtype=np.float32)
    consts[0:128, 0:128] = np.eye(128, dtype=np.float32)
    for p in range(16):
        consts[p, 128 + p:256:16] = 1.0
    identb = np.eye(128, dtype=np.float32).astype(bf16)

    in_maps = []
    for core in range(8):
        b, g = core // 2, core % 2
        cols = slice(g * HPC * HD, (g + 1) * HPC * HD)
        wq = np.ascontiguousarray(w_qkv[:, 0:C][:, cols])
        wqh = wq.astype(bf16)
        wql = (wq - wqh.astype(np.float32)).astype(bf16)
        xT = np.ascontiguousarray(x[b].T)
        xh = xT.astype(bf16)
        xl = (xT - xh.astype(np.float32)).astype(bf16)
        in_maps.append({
            "xh": xh,
            "xl": xl,
            "wqh": wqh,
            "wql": wql,
            "wk": np.ascontiguousarray(w_qkv[:, C:2 * C][:, cols]).astype(bf16),
            "wv": np.ascontiguousarray(w_qkv[:, 2 * C:3 * C][:, cols]).astype(bf16),
            "wp": np.ascontiguousarray(w_proj[cols, :]).astype(bf16),
            "selmask": selmask,
            "consts": consts,
            "identb": identb,
        })

    nc = _get_nc()
    r = run_bass_kernel_spmd(nc, in_maps, list(range(8)), trace=TRACE)
    LAST["exec_time_ns"] = r.exec_time_ns
    LAST["mean_exec_time_ns"] = r.mean_exec_time_ns
    LAST["results"] = r.results
    LAST["insts"] = r.instructions_and_trace
    y = np.empty((B, N, C), dtype=np.float32)
    for b in range(B):
        y[b] = (r.results[2 * b]["y"].astype(np.float32)
                + r.results[2 * b + 1]["y"].astype(np.float32))
    y = np.clip(y + b_proj, -10.0, 10.0)
    return y
